# revision 1
# baseline (speedup 1.0000x reference)
"""HardGateMOE Trainium2 kernel: expert-parallel across 8 NeuronCores.

Strategy: each core owns one expert (W1[e], W2[e]). The host performs the
"all-to-all token dispatch by mapping": for each expert it gathers the unique
tokens routed to it (padded to a common capacity C), transposed so the token
dim sits on the matmul free axis on device. Each core runs
  hT = gelu(W1[e].T @ xgT + b1)   # [F, C], tokens on free axis
  yT = W2[e].T @ hT + b2          # [H, C]
plus its 1/8 slice of the gate GEMM z = x @ Wg.T. The host then applies the
token-axis softmax, gathers per-(token,k) gate weights, and scatter-adds the
weighted expert outputs. Expert GEMMs run in bf16 (same PE rate as fp32r but
half the weight-DMA bytes); the gate GEMM runs in float32r so the softmax
weights stay ~1e-4 accurate. Only index bookkeeping, the tiny [N,E] softmax,
and the weighted combine run on host.
"""

import ml_dtypes
import numpy as np

import concourse.tile as tile
from concourse import bacc, mybir
from concourse.bass_utils import run_bass_kernel_spmd

N, H, F, E = 2048, 1024, 4096, 8
NCORES = 8
P = 128
NS = N // NCORES          # tokens per core for the gate GEMM slice
KH = H // P               # 8  k-chunks for fc1 (contract over H)
KF = F // P               # 32 k-chunks for fc2 (contract over F)
FT = F // P               # 32 f-tiles of hT
HT = H // P               # 8  h-tiles of yT
FG = 8                    # fc1 f-tile groups (4 f-tiles = 512 cols each)
GW = F // FG              # 512 cols of W1 per group
GF = GW // P              # 4 f-tiles per group

BF16 = ml_dtypes.bfloat16

_compiled = {}


def _build(C: int, repeats: int = 1):
    """Build + compile the SPMD program for token capacity C (<=512)."""
    nc = bacc.Bacc("TRN2", target_bir_lowering=False, debug=False,
                   num_devices=NCORES)
    f32 = mybir.dt.float32
    f32r = mybir.dt.float32r
    bf16 = mybir.dt.bfloat16
    GELU = mybir.ActivationFunctionType.Gelu_apprx_tanh
    IDENT = mybir.ActivationFunctionType.Identity

    xg = nc.dram_tensor("xg", [P, KH, C], bf16, kind="ExternalInput").ap()
    w1 = nc.dram_tensor("w1", [FG, P, KH, GW], bf16, kind="ExternalInput").ap()
    b1 = nc.dram_tensor("b1", [P, FT], f32, kind="ExternalInput").ap()
    w2 = nc.dram_tensor("w2", [KF, P, H], bf16, kind="ExternalInput").ap()
    b2 = nc.dram_tensor("b2", [P, HT], f32, kind="ExternalInput").ap()
    xs = nc.dram_tensor("xs", [P, KH, NS], f32r, kind="ExternalInput").ap()
    wg = nc.dram_tensor("wg", [P, KH, E], f32r, kind="ExternalInput").ap()
    y = nc.dram_tensor("y", [HT, P, C], f32, kind="ExternalOutput").ap()
    z = nc.dram_tensor("z", [E, NS], f32, kind="ExternalOutput").ap()

    with tile.TileContext(nc) as tc:
      for _rep in range(repeats):
        with (
            tc.tile_pool(name="singles", bufs=1) as singles,
            tc.tile_pool(name="w1p", bufs=3) as w1p,
            tc.tile_pool(name="w2p", bufs=10) as w2p,
            tc.tile_pool(name="outp", bufs=8) as outp,
        ):
            # PE warmup: dummy matmuls on a memset tile fill the startup DMA
            # window and release the HAM clock-gate before real work arrives.
            warm = singles.tile([P, 512], bf16)
            nc.vector.memset(warm, 0.0)
            with tc.tile_pool(name="psw", bufs=1, space="PSUM") as psw:
                wps = psw.tile([P, 512], f32)
                for _ in range(5):
                    nc.tensor.matmul(out=wps, lhsT=warm[:, 0:P], rhs=warm,
                                     start=True, stop=True)

            # Startup: fine-grained interleave of xg slices and group-0 w1
            # k-slices so the first accumulation chain starts ~immediately.
            xg_s = singles.tile([P, KH, C], bf16)
            w1_g0 = w1p.tile([P, KH, GW], bf16, name="w1g0", tag="w1")
            nc.sync.dma_start(out=xg_s[:, 0:1, :], in_=xg[:, 0:1, :])
            nc.sync.dma_start(out=w1_g0[:, 0, :], in_=w1[0, :, 0, :])
            nc.sync.dma_start(out=xg_s[:, 1:3, :], in_=xg[:, 1:3, :])
            for k in range(1, 3):
                nc.sync.dma_start(out=w1_g0[:, k, :], in_=w1[0, :, k, :])
            nc.sync.dma_start(out=xg_s[:, 3:5, :], in_=xg[:, 3:5, :])
            for k in range(3, 5):
                nc.sync.dma_start(out=w1_g0[:, k, :], in_=w1[0, :, k, :])
            nc.sync.dma_start(out=xg_s[:, 5:KH, :], in_=xg[:, 5:KH, :])
            for k in range(5, KH):
                nc.sync.dma_start(out=w1_g0[:, k, :], in_=w1[0, :, k, :])
            b1_s = singles.tile([P, FT], f32)
            nc.gpsimd.dma_start(out=b1_s, in_=b1)
            hT_s = singles.tile([P, FT, C], bf16)

            with tc.tile_pool(name="ps1", bufs=6, space="PSUM") as ps1:
                def fc1_group(fg, w1_t):
                    if w1_t is None:
                        w1_t = w1p.tile([P, KH, GW], bf16, name=f"w1g{fg}",
                                        tag="w1")
                        nc.sync.dma_start(out=w1_t[:, 0:4, :],
                                          in_=w1[fg, :, 0:4, :])
                        nc.sync.dma_start(out=w1_t[:, 4:KH, :],
                                          in_=w1[fg, :, 4:KH, :])
                    for fl in range(GF):
                        ft = fg * GF + fl
                        ps = ps1.tile([P, C], f32, tag="ps1", name="ps")
                        for k in range(KH):
                            nc.tensor.matmul(
                                out=ps,
                                lhsT=w1_t[:, k, fl * P:(fl + 1) * P],
                                rhs=xg_s[:, k, :],
                                start=(k == 0), stop=(k == KH - 1))
                        nc.scalar.activation(
                            out=hT_s[:, ft, :], in_=ps, func=GELU,
                            bias=b1_s[:, ft:ft + 1])

                fc1_group(0, w1_g0)
                for fg in range(1, FG):
                    fc1_group(fg, None)

            # Gate GEMM (f32r) between fc1 and fc2: HW DMA queues are idle
            # here and the PSUM bank it uses frees before fc2 needs all 8.
            b2_s = singles.tile([P, HT], f32)
            nc.gpsimd.dma_start(out=b2_s, in_=b2)
            wg_s = singles.tile([P, KH, E], f32r)
            xs_s = singles.tile([P, KH, NS], f32r)
            nc.sync.dma_start(out=wg_s, in_=wg)
            nc.sync.dma_start(out=xs_s, in_=xs)
            with tc.tile_pool(name="psg", bufs=1, space="PSUM") as psg:
                ps_z = psg.tile([E, NS], f32)
                for k in range(KH):
                    nc.tensor.matmul(out=ps_z, lhsT=wg_s[:, k, :],
                                     rhs=xs_s[:, k, :],
                                     start=(k == 0), stop=(k == KH - 1))
                z_s = outp.tile([E, NS], f32, tag="z")
                nc.scalar.activation(out=z_s, in_=ps_z,
                                     func=mybir.ActivationFunctionType.Copy)
                nc.sync.dma_start(out=z, in_=z_s)

            # fc2: 8 PSUM accumulators across the 32-step contraction over F;
            # the last TAILF steps run per-h so ACT + output DMA overlap PE.
            with tc.tile_pool(name="ps2", bufs=HT, space="PSUM") as ps2:
                ps_y = [ps2.tile([P, C], f32, tag="ps2", name=f"ps_y{h}")
                        for h in range(HT)]
                TAILF = 6
                w2_t = []
                for f in range(KF):
                    t = w2p.tile([P, H], bf16, tag="w2", name="w2t")
                    (nc.sync if f % 2 == 0 else nc.scalar).dma_start(
                        out=t, in_=w2[f])
                    w2_t.append(t)
                    if f < KF - TAILF:
                        for h in range(HT):
                            nc.tensor.matmul(
                                out=ps_y[h],
                                lhsT=t[:, h * P:(h + 1) * P],
                                rhs=hT_s[:, f, :],
                                start=(f == 0), stop=False)
                for h in range(HT):
                    for f in range(KF - TAILF, KF):
                        nc.tensor.matmul(
                            out=ps_y[h],
                            lhsT=w2_t[f][:, h * P:(h + 1) * P],
                            rhs=hT_s[:, f, :],
                            start=False, stop=(f == KF - 1))
                    o_t = outp.tile([P, C], f32, tag="y", name=f"o{h}")
                    nc.scalar.activation(out=o_t, in_=ps_y[h],
                                         func=IDENT,
                                         bias=b2_s[:, h:h + 1])
                    nc.sync.dma_start(out=y[h], in_=o_t)

    nc.compile()
    return nc


def kernel(**inputs) -> np.ndarray:
    x = np.ascontiguousarray(np.asarray(inputs["x"], dtype=np.float32))
    mapping = np.asarray(inputs["mapping"]).astype(np.int64)
    Wg = np.asarray(inputs["Wg"], dtype=np.float32)
    W1 = np.asarray(inputs["W1"], dtype=np.float32)
    b1 = np.asarray(inputs["b1"], dtype=np.float32)
    W2 = np.asarray(inputs["W2"], dtype=np.float32)
    b2 = np.asarray(inputs["b2"], dtype=np.float32)

    n, h = x.shape
    assert (n, h) == (N, H)

    # Host-side dispatch: unique tokens per expert (a token routed to the
    # same expert by both slots contributes once, with summed gate weight).
    token_lists = []
    for e in range(E):
        tl = np.nonzero((mapping == e).any(axis=1))[0]
        token_lists.append(tl)
    maxc = max(len(tl) for tl in token_lists)
    C = max(256, -(-maxc // 8) * 8)
    assert C <= 512, f"per-expert token count {maxc} exceeds single-chunk capacity"

    if C not in _compiled:
        _compiled[C] = _build(C)
    nc = _compiled[C]

    # wg host layout [P, KH, E]: wg[r, k, e] = Wg[e, k*128+r]
    wg_arr = np.ascontiguousarray(Wg.T.reshape(KH, P, E).transpose(1, 0, 2))
    in_maps = []
    for e in range(E):
        tl = token_lists[e]
        xgT = np.zeros((H, C), dtype=BF16)
        xgT[:, :len(tl)] = x[tl].T.astype(BF16)
        xsT = x[e * NS:(e + 1) * NS].T.reshape(KH, P, NS)
        in_maps.append({
            # [P, KH, C]: xg[r, k, c] = x[tl[c], k*128+r]
            "xg": np.ascontiguousarray(xgT.reshape(KH, P, C).transpose(1, 0, 2)),
            # [FG, P, KH, GW]: w1[fg, r, k, c] = W1[k*128+r, fg*512+c]
            "w1": np.ascontiguousarray(
                W1[e].reshape(KH, P, FG, GW).transpose(2, 1, 0, 3)).astype(BF16),
            "b1": np.ascontiguousarray(b1[e].reshape(FT, P).T),
            "w2": W2[e].reshape(KF, P, H).astype(BF16),
            "b2": np.ascontiguousarray(b2[e].reshape(HT, P).T),
            # [P, KH, NS]
            "xs": np.ascontiguousarray(xsT.transpose(1, 0, 2)),
            "wg": wg_arr,
        })

    res = run_bass_kernel_spmd(nc, in_maps, list(range(NCORES)))

    # Host combine: token-axis softmax gate, per-(token,k) weights, scatter-add.
    zf = np.empty((N, E), dtype=np.float32)
    for e in range(E):
        zf[e * NS:(e + 1) * NS, :] = res.results[e]["z"].T
    zf -= zf.max(axis=0, keepdims=True)
    ez = np.exp(zf)
    logits = ez / ez.sum(axis=0, keepdims=True)
    w = np.take_along_axis(logits, mapping, axis=1)
    w = w / w.sum(axis=1, keepdims=True)

    out = np.zeros((N, H), dtype=np.float32)
    for e in range(E):
        tl = token_lists[e]
        yT = res.results[e]["y"].reshape(H, -1)
        cw = (w[tl, 0] * (mapping[tl, 0] == e)
              + w[tl, 1] * (mapping[tl, 1] == e)).astype(np.float32)
        out[tl] += cw[:, None] * yT[:, :len(tl)].T
    return out



# revision 19
# speedup vs baseline: 1.3131x; 1.3131x over previous
"""HardGateMOE Trainium2 kernel: expert-parallel across 8 NeuronCores.

Strategy: each core owns one expert (W1[e], W2[e]). The host performs the
"all-to-all token dispatch by mapping": for each expert it gathers the unique
tokens routed to it (padded to a common capacity C), transposed so the token
dim sits on the matmul free axis on device. Each core runs
  hT = gelu(W1[e].T @ xgT + b1)   # [F, C], tokens on free axis
  yT = W2[e].T @ hT + b2          # [H, C]
plus its 1/8 slice of the gate GEMM z = x @ Wg.T. The host then applies the
token-axis softmax, gathers per-(token,k) gate weights, and scatter-adds the
weighted expert outputs. Expert GEMMs run in bf16 (same PE rate as fp32r but
half the weight-DMA bytes); the gate GEMM runs in float32r so the softmax
weights stay ~1e-4 accurate. Only index bookkeeping, the tiny [N,E] softmax,
and the weighted combine run on host.
"""

import ml_dtypes
import numpy as np

import concourse.tile as tile
from concourse import bacc, mybir
from concourse.bass_utils import run_bass_kernel_spmd

N, H, F, E = 2048, 1024, 4096, 8
NCORES = 8
P = 128
NS = N // NCORES          # tokens per core for the gate GEMM slice
KH = H // P               # 8  k-chunks for fc1 (contract over H)
KF = F // P               # 32 k-chunks for fc2 (contract over F)
FT = F // P               # 32 f-tiles of hT
HT = H // P               # 8  h-tiles of yT
FG = 8                    # fc1 f-tile groups (4 f-tiles = 512 cols each)
GW = F // FG              # 512 cols of W1 per group
GF = GW // P              # 4 f-tiles per group

BF16 = ml_dtypes.bfloat16

_compiled = {}


def _build(C: int, repeats: int = 1):
    """Build + compile the SPMD program for token capacity C (<=512)."""
    nc = bacc.Bacc("TRN2", target_bir_lowering=False, debug=False,
                   num_devices=NCORES)
    f32 = mybir.dt.float32
    f32r = mybir.dt.float32r
    bf16 = mybir.dt.bfloat16
    GELU = mybir.ActivationFunctionType.Gelu_apprx_tanh
    IDENT = mybir.ActivationFunctionType.Identity

    xg = nc.dram_tensor("xg", [P, KH, C], bf16, kind="ExternalInput").ap()
    w1 = nc.dram_tensor("w1", [FG, P, KH, GW], bf16, kind="ExternalInput").ap()
    b1 = nc.dram_tensor("b1", [P, FT], f32, kind="ExternalInput").ap()
    w2 = nc.dram_tensor("w2", [KF, P, H], bf16, kind="ExternalInput").ap()
    b2 = nc.dram_tensor("b2", [P, HT], f32, kind="ExternalInput").ap()
    xs = nc.dram_tensor("xs", [P, KH, NS], f32r, kind="ExternalInput").ap()
    wg = nc.dram_tensor("wg", [P, KH, E], f32r, kind="ExternalInput").ap()
    y = nc.dram_tensor("y", [HT, P, C], f32, kind="ExternalOutput").ap()
    z = nc.dram_tensor("z", [E, NS], f32, kind="ExternalOutput").ap()

    with tile.TileContext(nc) as tc:
      with (
          tc.tile_pool(name="singles", bufs=2) as singles,
          tc.tile_pool(name="w1p", bufs=3) as w1p,
          tc.tile_pool(name="w2p", bufs=10) as w2p,
          tc.tile_pool(name="outp", bufs=8) as outp,
      ):
        # SBUF pools live across repeats so iteration N+1's input DMAs
        # overlap iteration N's output drain (the per-repeat pool teardown
        # barrier previously serialized ~3us of start+drain per iteration).
        for _rep in range(repeats):
            if _rep == 0:
                # PE warmup: dummy matmuls on a memset tile fill the startup
                # DMA window and release the HAM clock-gate before real work
                # arrives. gpsimd's queue drains ~1us before DVE's at program
                # start, so memset there lets the warmup begin sooner.
                warm = singles.tile([P, 512], bf16, name="warm")
                nc.gpsimd.memset(warm, 0.0)
                with tc.tile_pool(name="psw", bufs=1, space="PSUM") as psw:
                    wps = psw.tile([P, 512], f32)
                    for _ in range(5):
                        nc.tensor.matmul(out=wps, lhsT=warm[:, 0:P],
                                         rhs=warm, start=True, stop=True)
            # Gate GEMM inputs go on the gpsimd SWDGE queue (never contends
            # with the sync-queue weight stream); wg is tiny and issues now,
            # xs (1 MB) issues after fc1 group 1 so it does not steal DMA
            # bandwidth from the startup-critical xg/w1 transfers. The gate
            # GEMM runs between fc1 groups (PSUM: 6 ps1 + 1 = 7 banks).
            wg_s = singles.tile([P, KH, E], f32r, name="wg")
            xs_s = singles.tile([P, KH, NS], f32r, name="xs")
            b2_s = singles.tile([P, HT], f32, name="b2")
            nc.gpsimd.dma_start(out=b2_s, in_=b2)

            # Startup: fine-grained interleave of xg slices and group-0 w1
            # k-slices so the first accumulation chain starts ~immediately.
            xg_s = singles.tile([P, KH, C], bf16, name="xg")
            w1_g0 = w1p.tile([P, KH, GW], bf16, name="w1g0", tag="w1")
            nc.sync.dma_start(out=xg_s[:, 0:1, :], in_=xg[:, 0:1, :])
            nc.sync.dma_start(out=w1_g0[:, 0, :], in_=w1[0, :, 0, :])
            nc.sync.dma_start(out=xg_s[:, 1:3, :], in_=xg[:, 1:3, :])
            nc.sync.dma_start(out=w1_g0[:, 1:3, :], in_=w1[0, :, 1:3, :])
            nc.sync.dma_start(out=xg_s[:, 3:5, :], in_=xg[:, 3:5, :])
            nc.sync.dma_start(out=w1_g0[:, 3:5, :], in_=w1[0, :, 3:5, :])
            nc.sync.dma_start(out=xg_s[:, 5:KH, :], in_=xg[:, 5:KH, :])
            nc.sync.dma_start(out=w1_g0[:, 5:KH, :], in_=w1[0, :, 5:KH, :])
            b1_s = singles.tile([P, FT], f32, name="b1")
            nc.gpsimd.dma_start(out=b1_s, in_=b1)
            hT_s = singles.tile([P, FT, C], bf16, name="hT")

            with tc.tile_pool(name="ps1", bufs=7, space="PSUM") as ps1:
                def fc1_group(fg, w1_t):
                    if w1_t is None:
                        w1_t = w1p.tile([P, KH, GW], bf16, name=f"w1g{fg}",
                                        tag="w1")
                        nc.sync.dma_start(out=w1_t[:, 0:4, :],
                                          in_=w1[fg, :, 0:4, :])
                        nc.sync.dma_start(out=w1_t[:, 4:KH, :],
                                          in_=w1[fg, :, 4:KH, :])
                    for fl in range(GF):
                        ft = fg * GF + fl
                        ps = ps1.tile([P, C], f32, tag="ps1", name="ps")
                        for k in range(KH):
                            nc.tensor.matmul(
                                out=ps,
                                lhsT=w1_t[:, k, fl * P:(fl + 1) * P],
                                rhs=xg_s[:, k, :],
                                start=(k == 0), stop=(k == KH - 1))
                        nc.scalar.activation(
                            out=hT_s[:, ft, :], in_=ps, func=GELU,
                            bias=b1_s[:, ft:ft + 1])

                # Groups 0-1 run k-outer so each arriving per-k DMA slice
                # immediately feeds 4 matmuls -- the PE never waits for a
                # full group's weights before its first chains move.
                def fc1_group_kouter(fg, w1_t):
                    ps_g = [ps1.tile([P, C], f32, tag="ps1",
                                     name=f"psk{fg}_{fl}")
                            for fl in range(GF)]
                    for k in range(KH):
                        for fl in range(GF):
                            nc.tensor.matmul(
                                out=ps_g[fl],
                                lhsT=w1_t[:, k, fl * P:(fl + 1) * P],
                                rhs=xg_s[:, k, :],
                                start=(k == 0), stop=(k == KH - 1))
                    for fl in range(GF):
                        ft = fg * GF + fl
                        nc.scalar.activation(
                            out=hT_s[:, ft, :], in_=ps_g[fl], func=GELU,
                            bias=b1_s[:, ft:ft + 1])

                fc1_group_kouter(0, w1_g0)
                w1_g1 = w1p.tile([P, KH, GW], bf16, name="w1g1", tag="w1")
                for k in range(KH):
                    nc.sync.dma_start(out=w1_g1[:, k, :], in_=w1[1, :, k, :])
                fc1_group_kouter(1, w1_g1)
                nc.gpsimd.dma_start(out=xs_s, in_=xs)
                fc1_group(2, None)
                fc1_group(3, None)

                # Gate GEMM (f32r) inside fc1: its inputs landed long ago via
                # the SWDGE queue, its PSUM bank coexists with ps1's six, and
                # its z copy retires on ACT well before fc2 opens all 8 banks
                # (previously fc2 stalled ~1.7us on this chain at the
                # fc1->fc2 boundary).
                with tc.tile_pool(name="psg", bufs=1, space="PSUM") as psg:
                    ps_z = psg.tile([E, NS], f32)
                    for k in range(KH):
                        nc.tensor.matmul(out=ps_z, lhsT=wg_s[:, k, :],
                                         rhs=xs_s[:, k, :],
                                         start=(k == 0), stop=(k == KH - 1))
                    z_s = outp.tile([E, NS], f32, tag="z")
                    nc.scalar.activation(out=z_s, in_=ps_z,
                                         func=mybir.ActivationFunctionType.Copy)
                    nc.gpsimd.dma_start(out=z, in_=z_s)

                for fg in range(4, FG):
                    fc1_group(fg, None)

            # fc2: 8 PSUM accumulators across the 32-step contraction over F;
            # the last TAILF steps run per-h so ACT + output DMA overlap PE.
            with tc.tile_pool(name="ps2", bufs=HT, space="PSUM") as ps2:
                ps_y = [ps2.tile([P, C], f32, tag="ps2", name=f"ps_y{h}")
                        for h in range(HT)]
                TAILF = 6
                w2_t = []
                for f in range(KF):
                    t = w2p.tile([P, H], bf16, tag="w2", name="w2t")
                    (nc.sync if f % 2 == 0 else nc.scalar).dma_start(
                        out=t, in_=w2[f])
                    w2_t.append(t)
                    if f < KF - TAILF:
                        for h in range(HT):
                            nc.tensor.matmul(
                                out=ps_y[h],
                                lhsT=t[:, h * P:(h + 1) * P],
                                rhs=hT_s[:, f, :],
                                start=(f == 0), stop=False)
                # Output DMA issues rotate across the SP/DVE/Pool queues:
                # serializing all eight on SP.SEQ (1.2us per issue) previously
                # delayed the final y tile past the last matmul by ~4us.
                # even h -> gpsimd SWDGE (cheap issue, latency hidden
                # mid-stream), odd h (incl. the final tile) -> SP HWDGE.
                yq = [nc.gpsimd, nc.sync]
                for h in range(HT):
                    for f in range(KF - TAILF, KF):
                        nc.tensor.matmul(
                            out=ps_y[h],
                            lhsT=w2_t[f][:, h * P:(h + 1) * P],
                            rhs=hT_s[:, f, :],
                            start=False, stop=(f == KF - 1))
                    o_t = outp.tile([P, C], f32, tag="y", name=f"o{h}")
                    nc.scalar.activation(out=o_t, in_=ps_y[h],
                                         func=IDENT,
                                         bias=b2_s[:, h:h + 1])
                    yq[h % 2].dma_start(out=y[h], in_=o_t)

    nc.compile()
    return nc


def kernel(**inputs) -> np.ndarray:
    x = np.ascontiguousarray(np.asarray(inputs["x"], dtype=np.float32))
    mapping = np.asarray(inputs["mapping"]).astype(np.int64)
    Wg = np.asarray(inputs["Wg"], dtype=np.float32)
    W1 = np.asarray(inputs["W1"], dtype=np.float32)
    b1 = np.asarray(inputs["b1"], dtype=np.float32)
    W2 = np.asarray(inputs["W2"], dtype=np.float32)
    b2 = np.asarray(inputs["b2"], dtype=np.float32)

    n, h = x.shape
    assert (n, h) == (N, H)

    # Host-side dispatch: unique tokens per expert (a token routed to the
    # same expert by both slots contributes once, with summed gate weight).
    token_lists = []
    for e in range(E):
        tl = np.nonzero((mapping == e).any(axis=1))[0]
        token_lists.append(tl)
    maxc = max(len(tl) for tl in token_lists)
    C = max(256, -(-maxc // 8) * 8)
    assert C <= 512, f"per-expert token count {maxc} exceeds single-chunk capacity"

    if C not in _compiled:
        _compiled[C] = _build(C)
    nc = _compiled[C]

    # wg host layout [P, KH, E]: wg[r, k, e] = Wg[e, k*128+r]
    wg_arr = np.ascontiguousarray(Wg.T.reshape(KH, P, E).transpose(1, 0, 2))
    in_maps = []
    for e in range(E):
        tl = token_lists[e]
        xgT = np.zeros((H, C), dtype=BF16)
        xgT[:, :len(tl)] = x[tl].T.astype(BF16)
        xsT = x[e * NS:(e + 1) * NS].T.reshape(KH, P, NS)
        in_maps.append({
            # [P, KH, C]: xg[r, k, c] = x[tl[c], k*128+r]
            "xg": np.ascontiguousarray(xgT.reshape(KH, P, C).transpose(1, 0, 2)),
            # [FG, P, KH, GW]: w1[fg, r, k, c] = W1[k*128+r, fg*512+c]
            "w1": np.ascontiguousarray(
                W1[e].reshape(KH, P, FG, GW).transpose(2, 1, 0, 3)).astype(BF16),
            "b1": np.ascontiguousarray(b1[e].reshape(FT, P).T),
            "w2": W2[e].reshape(KF, P, H).astype(BF16),
            "b2": np.ascontiguousarray(b2[e].reshape(HT, P).T),
            # [P, KH, NS]
            "xs": np.ascontiguousarray(xsT.transpose(1, 0, 2)),
            "wg": wg_arr,
        })

    res = run_bass_kernel_spmd(nc, in_maps, list(range(NCORES)))

    # Host combine: token-axis softmax gate, per-(token,k) weights, scatter-add.
    zf = np.empty((N, E), dtype=np.float32)
    for e in range(E):
        zf[e * NS:(e + 1) * NS, :] = res.results[e]["z"].T
    zf -= zf.max(axis=0, keepdims=True)
    ez = np.exp(zf)
    logits = ez / ez.sum(axis=0, keepdims=True)
    w = np.take_along_axis(logits, mapping, axis=1)
    w = w / w.sum(axis=1, keepdims=True)

    out = np.zeros((N, H), dtype=np.float32)
    for e in range(E):
        tl = token_lists[e]
        yT = res.results[e]["y"].reshape(H, -1)
        cw = (w[tl, 0] * (mapping[tl, 0] == e)
              + w[tl, 1] * (mapping[tl, 1] == e)).astype(np.float32)
        out[tl] += cw[:, None] * yT[:, :len(tl)].T
    return out



# revision 20
# speedup vs baseline: 1.3815x; 1.0521x over previous
"""HardGateMOE Trainium2 kernel: expert-parallel across 8 NeuronCores.

Strategy: each core owns one expert (W1[e], W2[e]). The host performs the
"all-to-all token dispatch by mapping": for each expert it gathers the unique
tokens routed to it (padded to a common capacity C), transposed so the token
dim sits on the matmul free axis on device. Each core runs
  hT = gelu(W1[e].T @ xgT + b1)   # [F, C], tokens on free axis
  yT = W2[e].T @ hT + b2          # [H, C]
plus its 1/8 slice of the gate GEMM z = x @ Wg.T. The host then applies the
token-axis softmax, gathers per-(token,k) gate weights, and scatter-adds the
weighted expert outputs. Expert GEMMs run in bf16 (same PE rate as fp32r but
half the weight-DMA bytes); the gate GEMM runs in float32r so the softmax
weights stay ~1e-4 accurate. Only index bookkeeping, the tiny [N,E] softmax,
and the weighted combine run on host.
"""

import ml_dtypes
import numpy as np

import concourse.tile as tile
from concourse import bacc, mybir
from concourse.bass_utils import run_bass_kernel_spmd

N, H, F, E = 2048, 1024, 4096, 8
NCORES = 8
P = 128
NS = N // NCORES          # tokens per core for the gate GEMM slice
KH = H // P               # 8  k-chunks for fc1 (contract over H)
KF = F // P               # 32 k-chunks for fc2 (contract over F)
FT = F // P               # 32 f-tiles of hT
HT = H // P               # 8  h-tiles of yT
FG = 8                    # fc1 f-tile groups (4 f-tiles = 512 cols each)
GW = F // FG              # 512 cols of W1 per group
GF = GW // P              # 4 f-tiles per group

BF16 = ml_dtypes.bfloat16

_compiled = {}


def _build(C: int, repeats: int = 1):
    """Build + compile the SPMD program for token capacity C (<=512)."""
    nc = bacc.Bacc("TRN2", target_bir_lowering=False, debug=False,
                   num_devices=NCORES)
    f32 = mybir.dt.float32
    f32r = mybir.dt.float32r
    bf16 = mybir.dt.bfloat16
    GELU = mybir.ActivationFunctionType.Gelu_apprx_tanh
    IDENT = mybir.ActivationFunctionType.Identity

    xg = nc.dram_tensor("xg", [P, KH, C], bf16, kind="ExternalInput").ap()
    w1 = nc.dram_tensor("w1", [FG, P, KH, GW], bf16, kind="ExternalInput").ap()
    b1 = nc.dram_tensor("b1", [P, FT], f32, kind="ExternalInput").ap()
    w2 = nc.dram_tensor("w2", [KF, P, H], bf16, kind="ExternalInput").ap()
    b2 = nc.dram_tensor("b2", [P, HT], f32, kind="ExternalInput").ap()
    xs = nc.dram_tensor("xs", [P, KH, NS], f32r, kind="ExternalInput").ap()
    wg = nc.dram_tensor("wg", [P, KH, E], f32r, kind="ExternalInput").ap()
    y = nc.dram_tensor("y", [HT, P, C], f32, kind="ExternalOutput").ap()
    z = nc.dram_tensor("z", [E, NS], f32, kind="ExternalOutput").ap()

    with tile.TileContext(nc) as tc:
      with (
          tc.tile_pool(name="singles", bufs=2) as singles,
          tc.tile_pool(name="w1p", bufs=3) as w1p,
          tc.tile_pool(name="w2p", bufs=10) as w2p,
          tc.tile_pool(name="outp", bufs=8) as outp,
      ):
        # SBUF pools live across repeats so iteration N+1's input DMAs
        # overlap iteration N's output drain (the per-repeat pool teardown
        # barrier previously serialized ~3us of start+drain per iteration).
        for _rep in range(repeats):
            if _rep == 0:
                # PE warmup: dummy matmuls on a memset tile fill the startup
                # DMA window and release the HAM clock-gate before real work
                # arrives. gpsimd's queue drains ~1us before DVE's at program
                # start, so memset there lets the warmup begin sooner.
                warm = singles.tile([P, 512], bf16, name="warm")
                nc.gpsimd.memset(warm, 0.0)
                with tc.tile_pool(name="psw", bufs=1, space="PSUM") as psw:
                    wps = psw.tile([P, 512], f32)
                    for _ in range(5):
                        nc.tensor.matmul(out=wps, lhsT=warm[:, 0:P],
                                         rhs=warm, start=True, stop=True)
            # Gate GEMM inputs go on the gpsimd SWDGE queue (never contends
            # with the sync-queue weight stream); wg is tiny and issues now,
            # xs (1 MB) issues after fc1 group 1 so it does not steal DMA
            # bandwidth from the startup-critical xg/w1 transfers. The gate
            # GEMM runs between fc1 groups (PSUM: 6 ps1 + 1 = 7 banks).
            wg_s = singles.tile([P, KH, E], f32r, name="wg")
            xs_s = singles.tile([P, KH, NS], f32r, name="xs")
            b2_s = singles.tile([P, HT], f32, name="b2")
            nc.gpsimd.dma_start(out=b2_s, in_=b2)

            # Startup: fine-grained interleave of xg slices and group-0 w1
            # k-slices so the first accumulation chain starts ~immediately.
            xg_s = singles.tile([P, KH, C], bf16, name="xg")
            w1_g0 = w1p.tile([P, KH, GW], bf16, name="w1g0", tag="w1")
            nc.sync.dma_start(out=xg_s[:, 0:1, :], in_=xg[:, 0:1, :])
            nc.sync.dma_start(out=w1_g0[:, 0, :], in_=w1[0, :, 0, :])
            nc.sync.dma_start(out=xg_s[:, 1:3, :], in_=xg[:, 1:3, :])
            nc.sync.dma_start(out=w1_g0[:, 1:3, :], in_=w1[0, :, 1:3, :])
            nc.sync.dma_start(out=xg_s[:, 3:5, :], in_=xg[:, 3:5, :])
            nc.sync.dma_start(out=w1_g0[:, 3:5, :], in_=w1[0, :, 3:5, :])
            nc.sync.dma_start(out=xg_s[:, 5:KH, :], in_=xg[:, 5:KH, :])
            nc.sync.dma_start(out=w1_g0[:, 5:KH, :], in_=w1[0, :, 5:KH, :])
            b1_s = singles.tile([P, FT], f32, name="b1")
            nc.gpsimd.dma_start(out=b1_s, in_=b1)
            hT_s = singles.tile([P, FT, C], bf16, name="hT")

            with tc.tile_pool(name="ps1", bufs=7, space="PSUM") as ps1:
                def fc1_group(fg, w1_t):
                    if w1_t is None:
                        w1_t = w1p.tile([P, KH, GW], bf16, name=f"w1g{fg}",
                                        tag="w1")
                        nc.sync.dma_start(out=w1_t[:, 0:4, :],
                                          in_=w1[fg, :, 0:4, :])
                        nc.sync.dma_start(out=w1_t[:, 4:KH, :],
                                          in_=w1[fg, :, 4:KH, :])
                    for fl in range(GF):
                        ft = fg * GF + fl
                        ps = ps1.tile([P, C], f32, tag="ps1", name="ps")
                        for k in range(KH):
                            nc.tensor.matmul(
                                out=ps,
                                lhsT=w1_t[:, k, fl * P:(fl + 1) * P],
                                rhs=xg_s[:, k, :],
                                start=(k == 0), stop=(k == KH - 1))
                        nc.scalar.activation(
                            out=hT_s[:, ft, :], in_=ps, func=GELU,
                            bias=b1_s[:, ft:ft + 1])

                # Groups 0-1 run k-outer so each arriving per-k DMA slice
                # immediately feeds 4 matmuls -- the PE never waits for a
                # full group's weights before its first chains move.
                def fc1_group_kouter(fg, w1_t):
                    ps_g = [ps1.tile([P, C], f32, tag="ps1",
                                     name=f"psk{fg}_{fl}")
                            for fl in range(GF)]
                    for k in range(KH):
                        for fl in range(GF):
                            nc.tensor.matmul(
                                out=ps_g[fl],
                                lhsT=w1_t[:, k, fl * P:(fl + 1) * P],
                                rhs=xg_s[:, k, :],
                                start=(k == 0), stop=(k == KH - 1))
                    for fl in range(GF):
                        ft = fg * GF + fl
                        nc.scalar.activation(
                            out=hT_s[:, ft, :], in_=ps_g[fl], func=GELU,
                            bias=b1_s[:, ft:ft + 1])

                fc1_group_kouter(0, w1_g0)
                w1_g1 = w1p.tile([P, KH, GW], bf16, name="w1g1", tag="w1")
                nc.sync.dma_start(out=w1_g1[:, 0:4, :], in_=w1[1, :, 0:4, :])
                nc.sync.dma_start(out=w1_g1[:, 4:KH, :],
                                  in_=w1[1, :, 4:KH, :])
                # Gate operands ride the sync HWDGE queue behind group 1's
                # weights: landed by ~13us, well before the gate fires after
                # group 3. (SWDGE descriptor generation for the same 1 MB
                # f32r tile blocked Pool.SEQ for ~36us -- never again.)
                nc.sync.dma_start(out=wg_s, in_=wg)
                nc.sync.dma_start(out=xs_s, in_=xs)
                fc1_group_kouter(1, w1_g1)
                fc1_group(2, None)
                fc1_group(3, None)

                # Gate GEMM (f32r) inside fc1: its inputs landed long ago via
                # the SWDGE queue, its PSUM bank coexists with ps1's six, and
                # its z copy retires on ACT well before fc2 opens all 8 banks
                # (previously fc2 stalled ~1.7us on this chain at the
                # fc1->fc2 boundary).
                with tc.tile_pool(name="psg", bufs=1, space="PSUM") as psg:
                    ps_z = psg.tile([E, NS], f32)
                    for k in range(KH):
                        nc.tensor.matmul(out=ps_z, lhsT=wg_s[:, k, :],
                                         rhs=xs_s[:, k, :],
                                         start=(k == 0), stop=(k == KH - 1))
                    z_s = outp.tile([E, NS], f32, tag="z")
                    nc.scalar.activation(out=z_s, in_=ps_z,
                                         func=mybir.ActivationFunctionType.Copy)
                    nc.gpsimd.dma_start(out=z, in_=z_s)

                for fg in range(4, FG):
                    fc1_group(fg, None)

            # fc2: 8 PSUM accumulators across the 32-step contraction over F;
            # the last TAILF steps run per-h so ACT + output DMA overlap PE.
            with tc.tile_pool(name="ps2", bufs=HT, space="PSUM") as ps2:
                ps_y = [ps2.tile([P, C], f32, tag="ps2", name=f"ps_y{h}")
                        for h in range(HT)]
                TAILF = 6
                w2_t = []
                for f in range(KF):
                    t = w2p.tile([P, H], bf16, tag="w2", name="w2t")
                    (nc.sync if f % 2 == 0 else nc.scalar).dma_start(
                        out=t, in_=w2[f])
                    w2_t.append(t)
                    if f < KF - TAILF:
                        for h in range(HT):
                            nc.tensor.matmul(
                                out=ps_y[h],
                                lhsT=t[:, h * P:(h + 1) * P],
                                rhs=hT_s[:, f, :],
                                start=(f == 0), stop=False)
                # Output DMA issues rotate across the SP/DVE/Pool queues:
                # serializing all eight on SP.SEQ (1.2us per issue) previously
                # delayed the final y tile past the last matmul by ~4us.
                # even h -> gpsimd SWDGE (cheap issue, latency hidden
                # mid-stream), odd h (incl. the final tile) -> SP HWDGE.
                yq = [nc.gpsimd, nc.sync]
                for h in range(HT):
                    for f in range(KF - TAILF, KF):
                        nc.tensor.matmul(
                            out=ps_y[h],
                            lhsT=w2_t[f][:, h * P:(h + 1) * P],
                            rhs=hT_s[:, f, :],
                            start=False, stop=(f == KF - 1))
                    o_t = outp.tile([P, C], f32, tag="y", name=f"o{h}")
                    nc.scalar.activation(out=o_t, in_=ps_y[h],
                                         func=IDENT,
                                         bias=b2_s[:, h:h + 1])
                    yq[h % 2].dma_start(out=y[h], in_=o_t)

    nc.compile()
    return nc


def kernel(**inputs) -> np.ndarray:
    x = np.ascontiguousarray(np.asarray(inputs["x"], dtype=np.float32))
    mapping = np.asarray(inputs["mapping"]).astype(np.int64)
    Wg = np.asarray(inputs["Wg"], dtype=np.float32)
    W1 = np.asarray(inputs["W1"], dtype=np.float32)
    b1 = np.asarray(inputs["b1"], dtype=np.float32)
    W2 = np.asarray(inputs["W2"], dtype=np.float32)
    b2 = np.asarray(inputs["b2"], dtype=np.float32)

    n, h = x.shape
    assert (n, h) == (N, H)

    # Host-side dispatch: unique tokens per expert (a token routed to the
    # same expert by both slots contributes once, with summed gate weight).
    token_lists = []
    for e in range(E):
        tl = np.nonzero((mapping == e).any(axis=1))[0]
        token_lists.append(tl)
    maxc = max(len(tl) for tl in token_lists)
    C = max(256, -(-maxc // 8) * 8)
    assert C <= 512, f"per-expert token count {maxc} exceeds single-chunk capacity"

    if C not in _compiled:
        _compiled[C] = _build(C)
    nc = _compiled[C]

    # wg host layout [P, KH, E]: wg[r, k, e] = Wg[e, k*128+r]
    wg_arr = np.ascontiguousarray(Wg.T.reshape(KH, P, E).transpose(1, 0, 2))
    in_maps = []
    for e in range(E):
        tl = token_lists[e]
        xgT = np.zeros((H, C), dtype=BF16)
        xgT[:, :len(tl)] = x[tl].T.astype(BF16)
        xsT = x[e * NS:(e + 1) * NS].T.reshape(KH, P, NS)
        in_maps.append({
            # [P, KH, C]: xg[r, k, c] = x[tl[c], k*128+r]
            "xg": np.ascontiguousarray(xgT.reshape(KH, P, C).transpose(1, 0, 2)),
            # [FG, P, KH, GW]: w1[fg, r, k, c] = W1[k*128+r, fg*512+c]
            "w1": np.ascontiguousarray(
                W1[e].reshape(KH, P, FG, GW).transpose(2, 1, 0, 3)).astype(BF16),
            "b1": np.ascontiguousarray(b1[e].reshape(FT, P).T),
            "w2": W2[e].reshape(KF, P, H).astype(BF16),
            "b2": np.ascontiguousarray(b2[e].reshape(HT, P).T),
            # [P, KH, NS]
            "xs": np.ascontiguousarray(xsT.transpose(1, 0, 2)),
            "wg": wg_arr,
        })

    res = run_bass_kernel_spmd(nc, in_maps, list(range(NCORES)))

    # Host combine: token-axis softmax gate, per-(token,k) weights, scatter-add.
    zf = np.empty((N, E), dtype=np.float32)
    for e in range(E):
        zf[e * NS:(e + 1) * NS, :] = res.results[e]["z"].T
    zf -= zf.max(axis=0, keepdims=True)
    ez = np.exp(zf)
    logits = ez / ez.sum(axis=0, keepdims=True)
    w = np.take_along_axis(logits, mapping, axis=1)
    w = w / w.sum(axis=1, keepdims=True)

    out = np.zeros((N, H), dtype=np.float32)
    for e in range(E):
        tl = token_lists[e]
        yT = res.results[e]["y"].reshape(H, -1)
        cw = (w[tl, 0] * (mapping[tl, 0] == e)
              + w[tl, 1] * (mapping[tl, 1] == e)).astype(np.float32)
        out[tl] += cw[:, None] * yT[:, :len(tl)].T
    return out

